# revision 10
# baseline (speedup 1.0000x reference)
"""Trainium2 Bass kernel for nn_DualAxisAggAttn (dual-axis aggregation attention).

Reference semantics per batch image x[C=256, H=64, W=64], twice (W axis then H axis):
  qkv = conv1x1(x) -> {q:[1], k:[C], v:[C]};  s = softmax_axis(q)
  ctx[c,a] = sum_r k*s;  out = x + sigmoid(v) * ctx_bcast;  y = conv1x1(out)

Distribution: data-parallel over batch (16 images -> 2 per NeuronCore x 8 cores).

v2 structure (vs the 129us baseline):
  - STAGE FOLD: stage H is linear in y_W before each nonlinearity, so the
    stage-W fusion conv folds into stage-H weights host-side:
      qkvH' = qkvH @ WfW,  Wff = WfH @ WfW,  biases folded likewise.
    Stage H then consumes xeffW = x + gW*ctxW directly and the stage-W
    fusion matmul (1/3 of all PE work) plus its PSUM evictions vanish.
  - key-path linearity: ctx = Wk @ (sum_r x*E) / S (key conv after reduction).
  - fp8 e4m3 DoubleRow matmuls for stage-W q/v convs: both k-tiles fused in
    one matmul at 0.5 cycles/row (4x vs 2 bf16 matmuls). Weights pre-scaled
    x16 to dodge fp8 subnormals; the 1/16 folds into the ACT scale.
  - per-chunk avg-pool for the stage-W reduction (w is innermost); the /64
    cancels exactly in xe/S. Stage-H reduction is an in-place halving tree.
  - final psum -> DRAM via direct DMA (fp32 y), no ACT eviction pass.
  - sigmoid via tanh ((1+tanh(v/2))/2) so exp+tanh share one ACT table set;
    the 0.5 folds into the ctx scale, the +1 into the g2 scalar_tensor_tensor.
  - stage-W g2 runs on the (otherwise idle) GpSimd/Pool queue; its inner
    stride-0 broadcast would run 1x on DVE anyway.
"""

import numpy as np
import ml_dtypes
from contextlib import ExitStack

import concourse.bass as bass
import concourse.bacc as bacc
import concourse.tile as tile
import concourse.mybir as mybir
from concourse.bass_utils import run_bass_kernel_spmd

F32 = mybir.dt.float32
BF16 = mybir.dt.bfloat16
F8 = mybir.dt.float8e4
AF = mybir.ActivationFunctionType
ALU = mybir.AluOpType
AX = mybir.AxisListType
DR = mybir.MatmulPerfMode.DoubleRow
NPBF = ml_dtypes.bfloat16
NPF8 = ml_dtypes.float8_e4m3

B, C, H, W = 16, 256, 64, 64
HW = H * W
NCORES = 8
BPC = B // NCORES
KT = 2
CH = 512
NCH = HW // CH
GRP = CH // 64
F8SCALE = 16.0

# experiment toggles
GP_G2W = False     # stage-W g2 on the GpSimd/Pool queue (else DVE)
GP_G2H = False     # stage-H g2 on the GpSimd/Pool queue (else DVE)
USE_POOL = False   # stage-W reduce via pool_avg (else halving tree)

_BUILD_CACHE = {}
LAST_RESULTS = None


def _build(flags):
    bvW0, bkW0, bqW0, bvH0, bkH0, bqH0, by0 = flags
    nc = bacc.Bacc(trn_type="TRN2", target_bir_lowering=False, debug=False)

    x8_d = nc.dram_tensor("x8", [BPC, KT, 128, HW], F8, kind="ExternalInput").ap()
    xbf_d = nc.dram_tensor("xbf", [BPC, KT, 128, HW], BF16, kind="ExternalInput").ap()
    statW_d = nc.dram_tensor("statW8", [128, KT, 3, 128], F8, kind="ExternalInput").ap()
    statH_d = nc.dram_tensor("statH", [128, KT, 3, 128], BF16, kind="ExternalInput").ap()
    wkW_d = nc.dram_tensor("wkW", [128, KT, 2, 128], BF16, kind="ExternalInput").ap()
    wkH_d = nc.dram_tensor("wkH", [128, KT, 2, 128], BF16, kind="ExternalInput").ap()
    fusA_d = nc.dram_tensor("fusA", [128, KT, 2, 128], BF16, kind="ExternalInput").ap()
    fusB_d = nc.dram_tensor("fusB", [128, KT, 2, 128], BF16, kind="ExternalInput").ap()
    bias_d = nc.dram_tensor("biases", [7, 2, 128], F32, kind="ExternalInput").ap()
    y_d = nc.dram_tensor("y", [BPC, C, HW], BF16, kind="ExternalOutput").ap()

    with tile.TileContext(nc) as tc, ExitStack() as ctx:
        wp = ctx.enter_context(tc.tile_pool(name="weights", bufs=1))
        px8 = ctx.enter_context(tc.tile_pool(name="x8", bufs=2))
        pxb = ctx.enter_context(tc.tile_pool(name="xb", bufs=2))
        pxe = ctx.enter_context(tc.tile_pool(name="xe", bufs=2))
        pT = ctx.enter_context(tc.tile_pool(name="T", bufs=2))
        pg = ctx.enter_context(tc.tile_pool(name="gate", bufs=2))
        pacc = ctx.enter_context(tc.tile_pool(name="acc", bufs=2))
        psm = ctx.enter_context(tc.tile_pool(name="small", bufs=8))
        pch = ctx.enter_context(tc.tile_pool(name="chunk", bufs=2))
        phv = ctx.enter_context(tc.tile_pool(name="hv", bufs=1))
        pyv = ctx.enter_context(tc.tile_pool(name="yev", bufs=2))
        pq = ctx.enter_context(tc.tile_pool(name="psq", bufs=2, space="PSUM"))
        pvf = ctx.enter_context(tc.tile_pool(name="psvf", bufs=3, space="PSUM"))

        def wload(name, dram, shape, dt):
            t = wp.tile(shape, dt, tag=name)
            nc.scalar.dma_start(t[:], dram[:])
            return t

        statW = wload("statW8", statW_d, [128, KT, 3, 128], F8)
        statH = wload("statH", statH_d, [128, KT, 3, 128], BF16)
        wkW = wload("wkW", wkW_d, [128, KT, 2, 128], BF16)
        wkH = wload("wkH", wkH_d, [128, KT, 2, 128], BF16)
        fusA = wload("fusA", fusA_d, [128, KT, 2, 128], BF16)
        fusB = wload("fusB", fusB_d, [128, KT, 2, 128], BF16)

        bias_sb = wp.tile([128, 7, 2], F32, tag="biases")
        nc.scalar.dma_start(bias_sb[:], bias_d[:].transpose([2, 0, 1]))
        zb = wp.tile([128, 1], F32, tag="zb")
        nc.vector.memset(zb[:], 0.0)

        def bap(i, ct):
            return bias_sb[:, i, ct].unsqueeze(1)

        def load_x(b):
            x8t = px8.tile([128, KT, HW], F8, tag="x8")
            xbt = pxb.tile([128, KT, HW], BF16, tag="xb")
            for t, d in ((x8t, x8_d), (xbt, xbf_d)):
                for half in range(2):
                    hs = bass.ts(half, HW // 2)
                    nc.sync.dma_start(t[:, :, hs], d[b][:, :, hs].transpose([1, 0, 2]))
            return x8t, xbt

        def p1W(x8t, xbt, TW, gate, accW):
            bq = zb[:] if bqW0 else bap(4, 0)
            for j in range(NCH):
                sl = bass.ts(j, CH)
                rhs = x8t[:, :, sl]
                ps_q = pq.tile([128, CH], F32, tag="q")
                ps_v = pvf.tile([128, 2 * CH], F32, tag="vf")
                nc.tensor.matmul(ps_q[:], statW[:, :, 2, :], rhs, start=True, stop=True, perf_mode=DR)
                nc.tensor.matmul(ps_v[:, 0:CH], statW[:, :, 0, :], rhs, start=True, stop=True, perf_mode=DR)
                nc.tensor.matmul(ps_v[:, CH:], statW[:, :, 1, :], rhs, start=True, stop=True, perf_mode=DR)
                nc.scalar.activation(TW[:, 0, sl], ps_q[:], AF.Exp, bias=bq, scale=1.0 / F8SCALE)
                if bvW0:
                    nc.scalar.activation(
                        gate[:, :, sl], ps_v[:].rearrange("p (c n) -> p c n", c=2),
                        AF.Tanh, bias=zb[:], scale=0.5 / F8SCALE,
                    )
                else:
                    for ct in range(2):
                        nc.scalar.activation(
                            gate[:, ct, sl], ps_v[:, bass.ts(ct, CH)],
                            AF.Tanh, bias=bap(0, ct), scale=0.5 / F8SCALE,
                        )
                for ct in range(2):
                    nc.vector.tensor_tensor(TW[:, 1 + ct, sl], xbt[:, ct, sl], TW[:, 0, sl], op=ALU.mult)
                v4 = TW[:, :, sl].rearrange("p c (a r) -> p c a r", r=64)
                if USE_POOL:
                    nc.vector.pool_avg(accW[:, :, bass.ts(j, GRP)], v4)
                else:
                    hv = phv.tile([128, 3, GRP, 32], BF16, tag="hv")
                    nc.vector.tensor_tensor(hv[:], v4[:, :, :, 0:32], v4[:, :, :, 32:64], op=ALU.add)
                    nc.vector.tensor_tensor(hv[:, :, :, 0:16], hv[:, :, :, 0:16], hv[:, :, :, 16:32], op=ALU.add)
                    nc.vector.tensor_tensor(hv[:, :, :, 0:8], hv[:, :, :, 0:8], hv[:, :, :, 8:16], op=ALU.add)
                    nc.vector.tensor_reduce(accW[:, :, bass.ts(j, GRP)], hv[:, :, :, 0:8], axis=AX.X, op=ALU.add)

        def p2(acc, wk, bk0, bkrow, tag):
            R = psm.tile([128, 64], F32, tag=f"R{tag}")
            nc.vector.reciprocal(R[:], acc[:, 0, :])
            xn = psm.tile([128, 2, 64], BF16, tag=f"xn{tag}")
            nc.vector.tensor_tensor(
                xn[:], acc[:, 1:3, :], R[:].unsqueeze(1).broadcast_to([128, 2, 64]), op=ALU.mult
            )
            cns = []
            for mt in range(2):
                ps_c = pq.tile([128, 64], F32, tag="q")
                for ct in range(2):
                    nc.tensor.matmul(ps_c[:], wk[:, ct, mt, :], xn[:, ct, :], start=ct == 0, stop=ct == 1)
                cn = psm.tile([128, 64], BF16, tag=f"cn{tag}{mt}")
                if bk0:
                    nc.vector.tensor_scalar_mul(cn[:], ps_c[:], 0.5)
                else:
                    nc.vector.tensor_scalar(cn[:], ps_c[:], 0.5, bap(bkrow, mt), op0=ALU.mult, op1=ALU.add)
                cns.append(cn)
            return cns

        def p3W(xbt, gate, cns, xeff):
            eng = nc.gpsimd if GP_G2W else nc.vector
            for j in range(NCH):
                sl = bass.ts(j, CH)
                g2 = pch.tile([128, 2, GRP, 64], BF16, tag="g2w")
                for ct in range(2):
                    cb = cns[ct][:, bass.ts(j, GRP)].unsqueeze(2).broadcast_to([128, GRP, 64])
                    eng.scalar_tensor_tensor(
                        g2[:, ct], gate[:, ct, sl].rearrange("p (a r) -> p a r", r=64),
                        1.0, cb, op0=ALU.add, op1=ALU.mult,
                    )
                nc.vector.tensor_tensor(
                    xeff[:, :, sl], xbt[:, :, sl],
                    g2[:].rearrange("p c a r -> p c (a r)"), op=ALU.add,
                )

        def p1H(xeff, TH, gate):
            bq = zb[:] if bqH0 else bap(5, 0)
            for j in range(NCH):
                sl = bass.ts(j, CH)
                ps_q = pq.tile([128, CH], F32, tag="q")
                ps_v = pvf.tile([128, 2 * CH], F32, tag="vf")
                for kt in range(KT):
                    st, sp = kt == 0, kt == KT - 1
                    rhs = xeff[:, kt, sl]
                    nc.tensor.matmul(ps_q[:], statH[:, kt, 2, :], rhs, start=st, stop=sp)
                    nc.tensor.matmul(ps_v[:, 0:CH], statH[:, kt, 0, :], rhs, start=st, stop=sp)
                    nc.tensor.matmul(ps_v[:, CH:], statH[:, kt, 1, :], rhs, start=st, stop=sp)
                nc.scalar.activation(TH[:, 0, sl], ps_q[:], AF.Exp, bias=bq)
                if bvH0:
                    nc.scalar.activation(
                        gate[:, :, sl], ps_v[:].rearrange("p (c n) -> p c n", c=2),
                        AF.Tanh, bias=zb[:], scale=0.5,
                    )
                else:
                    for ct in range(2):
                        nc.scalar.activation(
                            gate[:, ct, sl], ps_v[:, bass.ts(ct, CH)],
                            AF.Tanh, bias=bap(2, ct), scale=0.5,
                        )
                for ct in range(2):
                    nc.vector.tensor_tensor(TH[:, 1 + ct, sl], xeff[:, ct, sl], TH[:, 0, sl], op=ALU.mult)

        def rtreeH(TH, accH):
            n = HW // 2
            while n >= 128:
                nc.vector.tensor_tensor(TH[:, :, 0:n], TH[:, :, 0:n], TH[:, :, n : 2 * n], op=ALU.add)
                n //= 2
            nc.vector.tensor_tensor(accH[:, :, :], TH[:, :, 0:64], TH[:, :, 64:128], op=ALU.add)

        def p3H(b, xeff, gate, cns):
            ydst = y_d[b].rearrange("(m p) n -> p m n", p=128)
            for j in range(NCH):
                sl = bass.ts(j, CH)
                g2 = pch.tile([128, 2, GRP, 64], BF16, tag="g2h")
                eng = nc.gpsimd if GP_G2H else nc.vector
                for ct in range(2):
                    cb = cns[ct][:].unsqueeze(1).broadcast_to([128, GRP, 64])
                    eng.scalar_tensor_tensor(
                        g2[:, ct], gate[:, ct, sl].rearrange("p (a r) -> p a r", r=64),
                        1.0, cb, op0=ALU.add, op1=ALU.mult,
                    )
                ps_f = pvf.tile([128, 2 * CH], F32, tag="vf")
                g2f = g2[:].rearrange("p c a r -> p c (a r)")
                for mt in range(2):
                    half = ps_f[:, bass.ts(mt, CH)]
                    nc.tensor.matmul(half, fusA[:, 0, mt, :], xeff[:, 0, sl], start=True, stop=False)
                    nc.tensor.matmul(half, fusA[:, 1, mt, :], xeff[:, 1, sl], start=False, stop=False)
                    nc.tensor.matmul(half, fusB[:, 0, mt, :], g2f[:, 0], start=False, stop=False)
                    nc.tensor.matmul(half, fusB[:, 1, mt, :], g2f[:, 1], start=False, stop=True)
                y_t = pyv.tile([128, 2, CH], BF16, tag="y")
                if by0:
                    # alternate psum eviction between ACT and DVE (gpsimd
                    # cannot read PSUM)
                    if j % 2 == 0:
                        nc.scalar.activation(
                            y_t[:], ps_f[:].rearrange("p (m n) -> p m n", m=2), AF.Copy
                        )
                    else:
                        nc.vector.tensor_copy(
                            y_t[:], ps_f[:].rearrange("p (m n) -> p m n", m=2)
                        )
                else:
                    for mt in range(2):
                        nc.scalar.activation(
                            y_t[:, mt, :], ps_f[:, bass.ts(mt, CH)],
                            AF.Identity, bias=bap(6, mt),
                        )
                nc.sync.dma_start(ydst[:, :, sl], y_t[:])

        # ---- schedule: 2 images, stage phases interleaved ----
        x80, xb0 = load_x(0)
        x81, xb1 = load_x(1)

        TW0 = pT.tile([128, 3, HW], BF16, tag="T")
        gW0 = pg.tile([128, 2, HW], BF16, tag="gate")
        aW0 = pacc.tile([128, 3, 64], F32, tag="acc")
        p1W(x80, xb0, TW0, gW0, aW0)

        TW1 = pT.tile([128, 3, HW], BF16, tag="T")
        gW1 = pg.tile([128, 2, HW], BF16, tag="gate")
        aW1 = pacc.tile([128, 3, 64], F32, tag="acc")
        p1W(x81, xb1, TW1, gW1, aW1)

        cnsW0 = p2(aW0, wkW, bkW0, 1, "W0")
        xe0 = pxe.tile([128, KT, HW], BF16, tag="xe")
        p3W(xb0, gW0, cnsW0, xe0)

        cnsW1 = p2(aW1, wkW, bkW0, 1, "W1")

        TH0 = pT.tile([128, 3, HW], BF16, tag="T")
        gH0 = pg.tile([128, 2, HW], BF16, tag="gate")
        p1H(xe0, TH0, gH0)

        xe1 = pxe.tile([128, KT, HW], BF16, tag="xe")
        p3W(xb1, gW1, cnsW1, xe1)

        aH0 = pacc.tile([128, 3, 64], F32, tag="acc")
        rtreeH(TH0, aH0)
        cnsH0 = p2(aH0, wkH, bkH0, 3, "H0")

        TH1 = pT.tile([128, 3, HW], BF16, tag="T")
        gH1 = pg.tile([128, 2, HW], BF16, tag="gate")
        p1H(xe1, TH1, gH1)

        p3H(0, xe0, gH0, cnsH0)

        aH1 = pacc.tile([128, 3, 64], F32, tag="acc")
        rtreeH(TH1, aH1)
        cnsH1 = p2(aH1, wkH, bkH0, 3, "H1")

        p3H(1, xe1, gH1, cnsH1)

    nc.compile()
    return nc


def _stat_np(qkv_w):
    wq = qkv_w[0]
    wk = qkv_w[1 : 1 + C]
    wv = qkv_w[1 + C :]
    stat = np.empty((128, KT, 3, 128), np.float64)
    wkt = np.empty((128, KT, 2, 128), np.float64)
    for kt in range(KT):
        cs = slice(kt * 128, (kt + 1) * 128)
        stat[:, kt, 0, :] = wv[0:128, cs].T
        stat[:, kt, 1, :] = wv[128:256, cs].T
        stat[:, kt, 2, :] = np.repeat(wq[cs][:, None], 128, axis=1)
        wkt[:, kt, 0, :] = wk[0:128, cs].T
        wkt[:, kt, 1, :] = wk[128:256, cs].T
    return stat, wkt


def _fus_np(fw):
    fus = np.empty((128, KT, 2, 128), np.float64)
    for kt in range(KT):
        cs = slice(kt * 128, (kt + 1) * 128)
        fus[:, kt, 0, :] = fw[0:128, cs].T
        fus[:, kt, 1, :] = fw[128:256, cs].T
    return fus


def kernel(x, qkvW_w, qkvW_b, qkvH_w, qkvH_b, fusW_w, fusW_b, fusH_w, fusH_b):
    global LAST_RESULTS
    f64 = np.float64
    x = np.asarray(x, np.float32)
    qW = np.asarray(qkvW_w, f64)
    bW = np.asarray(qkvW_b, f64)
    qH = np.asarray(qkvH_w, f64)
    bH = np.asarray(qkvH_b, f64)
    fW = np.asarray(fusW_w, f64)
    fWb = np.asarray(fusW_b, f64)
    fH = np.asarray(fusH_w, f64)
    fHb = np.asarray(fusH_b, f64)

    # stage fold: stage H consumes xeffW directly
    qHf = qH @ fW
    bHf = qH @ fWb + bH
    Wff = fH @ fW
    b_y = fH @ fWb + fHb

    statW, wkW = _stat_np(qW)
    statH, wkH = _stat_np(qHf)
    fusA = _fus_np(Wff)
    fusB = _fus_np(fH)

    tobf = lambda a: np.ascontiguousarray(a.astype(np.float32).astype(NPBF))
    statW8 = np.ascontiguousarray((statW * F8SCALE).astype(np.float32).astype(NPF8))
    statH16 = tobf(statH)
    wkW16 = tobf(wkW)
    wkH16 = tobf(wkH)
    fusA16 = tobf(fusA)
    fusB16 = tobf(fusB)

    bqW, bkW, bvW = bW[0], bW[1 : 1 + C], bW[1 + C :]
    bqH, bkH, bvH = bHf[0], bHf[1 : 1 + C], bHf[1 + C :]
    biases = np.stack(
        [
            (0.5 * bvW).reshape(2, 128),
            (0.5 * bkW).reshape(2, 128),
            (0.5 * bvH).reshape(2, 128),
            (0.5 * bkH).reshape(2, 128),
            np.full((2, 128), bqW),
            np.full((2, 128), bqH),
            b_y.reshape(2, 128),
        ]
    ).astype(np.float32)

    flags = tuple(not np.any(a) for a in (bvW, bkW, bqW, bvH, bkH, bqH, b_y))
    if flags not in _BUILD_CACHE:
        _BUILD_CACHE[flags] = _build(flags)
    nc = _BUILD_CACHE[flags]

    x4 = x.reshape(B, KT, 128, HW)
    x8 = np.ascontiguousarray(x4.astype(NPF8))
    xb = np.ascontiguousarray(x4.astype(NPBF))
    in_maps = []
    for core in range(NCORES):
        bs = slice(core * BPC, (core + 1) * BPC)
        in_maps.append(
            {
                "x8": x8[bs],
                "xbf": xb[bs],
                "statW8": statW8,
                "statH": statH16,
                "wkW": wkW16,
                "wkH": wkH16,
                "fusA": fusA16,
                "fusB": fusB16,
                "biases": biases,
            }
        )

    res = run_bass_kernel_spmd(nc, in_maps, list(range(NCORES)))
    LAST_RESULTS = res
    y = np.concatenate([r["y"] for r in res.results], axis=0)
    return np.ascontiguousarray(y.astype(np.float32).reshape(B, C, H, W))
